# revision 36
# baseline (speedup 1.0000x reference)
"""Causal self-attention kernel for 8 trn2 NeuronCores.

Sharding: 4 batches x 2 head-groups (8 heads each). Core c handles
batch c//2, heads (c%2)*8 .. (c%2)*8+8. Each core computes qkv for its
head-group, causal attention, and a partial projection; the host sums
the two head-group partials per batch and adds b_proj.

v3 design notes:
  - bf16 operands end-to-end (PSUM accumulate stays fp32).
  - fine-grained causal: diagonal 128-key tiles only compute the valid
    query suffix (scores, exp, PV all shrink); one 128x128 triangle
    mask handles the partial block.
  - attention jt-loop is exp(ScalarE)-paced, so the next head-pair's
    q/k projection matmuls (and, on the last pair, the output
    projection) are interleaved into it via deferred-work generators.
  - ScalarE runs ONLY exp; q/k PSUM evac is VectorE tensor_scalar_add,
    projection evac is VectorE copy.
  - the per-chunk softmax normalization (reciprocal/broadcast/mul) is
    emitted one chunk late so its DMA-latency chain never blocks the
    in-order Vector/GpSimd queues between chunks.
  - inputs arrive as a handful of large strided DMAs (the SP queue
    spends ~0.6us issuing each DMA, so descriptor count matters).
"""

import sys
import os

for _p in ("/opt/trn_rl_repo", "/root/.axon_site/_ro/trn_rl_repo"):
    if os.path.isdir(_p) and _p not in sys.path:
        sys.path.insert(0, _p)

import numpy as np
import ml_dtypes
import concourse.bass as bass  # noqa: F401
import concourse.mybir as mybir
import concourse.tile as tile
from concourse import bacc, bass_utils

F32 = mybir.dt.float32
F32R = mybir.dt.float32r
BF16 = mybir.dt.bfloat16
ActF = mybir.ActivationFunctionType

B, S, D, H = 4, 2048, 1024, 16
NH = 8          # heads per core
HPAIRS = NH // 2
KT = D // 128   # 8 k-tiles over D
N_CORES = 8

_nc_cache = {}


def build_nc(S_tok=S, n_cores=N_CORES):
    key = (S_tok, n_cores)
    if key in _nc_cache:
        return _nc_cache[key]
    IC = S_tok // 512      # query chunks
    NT = S_tok // 128      # token tiles
    nc = bacc.Bacc("TRN2", target_bir_lowering=False, debug=False,
                   num_devices=n_cores)
    # host pre-arranges x and weights into the on-chip layouts so every
    # input DMA is a long-contiguous-row copy (8KB runs)
    NCH = S_tok // 512
    xT = nc.dram_tensor("xT", [128, NCH, KT, 512], BF16,
                        kind="ExternalInput").ap()
    Wq = nc.dram_tensor("Wq", [128, KT, 512], BF16,
                        kind="ExternalInput").ap()
    Wk = nc.dram_tensor("Wk", [128, KT, 512], BF16,
                        kind="ExternalInput").ap()
    Wv = nc.dram_tensor("Wv", [128, KT, 512], BF16,
                        kind="ExternalInput").ap()
    Wb = nc.dram_tensor("Wb", [3, 512], F32, kind="ExternalInput").ap()
    Wp = nc.dram_tensor("Wp", [128, HPAIRS, D], BF16,
                        kind="ExternalInput").ap()
    out = nc.dram_tensor("out", [S_tok, D], BF16, kind="ExternalOutput").ap()

    with tile.TileContext(nc) as tc:
        with tc.tile_pool(name="persist", bufs=1) as pp:
            # big resident tiles; [p, k, cols] so one strided DMA loads all
            xtr = pp.tile([128, NCH, KT, 512], BF16, name="xtr")
            wvb = pp.tile([128, KT, 512], BF16, name="wvb")
            wqb = pp.tile([128, KT, 512], BF16, name="wqb")
            wkb = pp.tile([128, KT, 512], BF16, name="wkb")
            wpb = pp.tile([128, HPAIRS, D], BF16, name="wpb")
            # v in natural layout, 65-stride per head (64 v cols + ones col)
            v_sb = [pp.tile([128, 8 * 65], BF16, name=f"vsb{t}")
                    for t in range(NT)]
            # normalized attention output per head pair [local d, tokens]
            yT = [pp.tile([128, S_tok], BF16, name=f"ytr{h}")
                  for h in range(HPAIRS)]
            # triangle mask: m[p, y] = 1 if y >= p else 0
            mask = pp.tile([128, 128], BF16, name="mask")
            # rank-1 stationary for broadcasting 1/Z rows across partitions
            ones64 = pp.tile([1, 64], F32, name="ones64")
            nc.gpsimd.memset(ones64, 1.0)
            nc.gpsimd.memset(mask, 1.0)
            nc.gpsimd.affine_select(
                out=mask, in_=mask, compare_op=mybir.AluOpType.is_ge,
                fill=0.0, base=0, pattern=[[1, 128]],
                channel_multiplier=-1)
            # softmax-denominator ones columns, written once
            for t in range(NT):
                nc.gpsimd.memset(
                    v_sb[t].rearrange("p (h c) -> p h c", c=65)[:, :, 64:65],
                    1.0)

            # ---- input DMAs: host-prearranged layouts make every load
            # a contiguous-row copy (8KB runs); chunk-0 of x and the V
            # weights first since they gate the first matmul; the v bias
            # row early so its broadcast never blocks the V evacs ----
            nc.sync.dma_start(wqb, Wq)
            nc.sync.dma_start(xtr[:, 0, :, :], xT[:, 0, :, :])
            nc.sync.dma_start(wkb, Wk)
            nc.sync.dma_start(xtr[:, 1, :, :], xT[:, 1, :, :])
            nc.sync.dma_start(wvb, Wv)
            bvr = pp.tile([1, 512], F32, name="bvr")
            nc.sync.dma_start(bvr, Wb[2:3, :])
            bvb = pp.tile([128, 512], F32, name="bvb")
            nc.gpsimd.partition_broadcast(bvb, bvr)
            wq9, wk9 = [], []
            for hp in range(HPAIRS):
                hs = slice(hp * 128, (hp + 1) * 128)
                t9 = pp.tile([128, 1], F32, name=f"wq9_{hp}")
                nc.sync.dma_start(t9, Wb[0:1, hs])
                wq9.append(t9)
                t9 = pp.tile([128, 1], F32, name=f"wk9_{hp}")
                nc.sync.dma_start(t9, Wb[1:2, hs])
                wk9.append(t9)
            nc.sync.dma_start(xtr[:, 2, :, :], xT[:, 2, :, :])
            nc.sync.dma_start(xtr[:, 3, :, :], xT[:, 3, :, :])
            nc.sync.dma_start(wpb, Wp)

            # ---- attention with V / q-k / projection all flowing through
            # a deadline-enforced deferred-work queue ----
            with tc.tile_pool(name="hsb", bufs=1) as hsb, \
                 tc.tile_pool(name="ps", bufs=1, space="PSUM") as ps:

                def v_group(g):
                    """Generator: V projection + bias for token tiles
                    4g..4g+3, one [128,512] psq slot per token tile
                    (bufs=2 hides the evac latency)."""
                    for t in range(g * 4, g * 4 + 4):
                        psv = ps.tile([128, 512], F32, tag="psq", bufs=2,
                                      name="psv")
                        for k in range(KT):
                            nc.tensor.matmul(
                                psv, xtr[:, t // 4, k,
                                     (t % 4) * 128:(t % 4) * 128 + 128],
                                wvb[:, k, :], start=(k == 0),
                                stop=(k == KT - 1))
                            yield
                        vv = v_sb[t].rearrange("p (h c) -> p h c", c=65)
                        nc.vector.tensor_add(
                            vv[:, :, 0:64],
                            psv.rearrange("p (h c) -> p h c", c=64),
                            bvb.rearrange("p (h c) -> p h c", c=64))

                # all four head-pairs' q/k stay live (bf16 keeps it small)
                qts = [hsb.tile([128, S_tok], BF16, name=f"qt{h}")
                       for h in range(HPAIRS)]
                kts = [hsb.tile([128, S_tok], BF16, name=f"kt{h}")
                       for h in range(HPAIRS)]

                def qk_gen(hp, qt, kt_t, half):
                    """Generator: q/k projection (one token half) for
                    head-pair hp; evac on VectorE with per-partition
                    bias, one [128,512] psq slot per 512-token chunk."""
                    hs = slice(hp * 128, (hp + 1) * 128)
                    for dst, wb_, w9 in ((qt, wqb, wq9[hp]),
                                         (kt_t, wkb, wk9[hp])):
                        for sub in range(2):
                            ch = half * 2 + sub
                            psq = ps.tile([128, 512], F32, tag="psq",
                                          bufs=2, name="psq")
                            for k in range(KT):
                                nc.tensor.matmul(
                                    psq, wb_[:, k, hs],
                                    xtr[:, ch, k, :],
                                    start=(k == 0), stop=(k == KT - 1))
                                yield
                            nc.vector.tensor_scalar_add(
                                dst[:, ch * 512:(ch + 1) * 512], psq, w9)

                def proj_chunk(ic):
                    """Generator: output projection for token tiles of
                    query chunk ic; needs yT[*][:, ic*512:(ic+1)*512]."""
                    for tt in range(ic * 4, ic * 4 + 4):
                        ot = hsb.tile([128, 1024], BF16, tag="ot", bufs=3,
                                      name="ot")
                        for nch in range(2):
                            pso = ps.tile([128, 512], F32, tag="psq",
                                          bufs=2, name="pso")
                            for k in range(HPAIRS):
                                nc.tensor.matmul(
                                    pso,
                                    yT[k][:, tt * 128:(tt + 1) * 128],
                                    wpb[:, k, nch * 512:(nch + 1) * 512],
                                    start=(k == 0), stop=(k == HPAIRS - 1))
                                yield
                            nc.vector.tensor_copy(
                                ot[:, nch * 512:(nch + 1) * 512], pso)
                        nc.sync.dma_start(out[tt * 128:(tt + 1) * 128, :], ot)

                proj3 = {"parts": []}

                def proj3_pre():
                    """Generator: last-chunk projection partials over
                    head-pairs 0..2 (hp3's yT is not ready yet); runs
                    during the last cell's attention."""
                    for tt in range((IC - 1) * 4, (IC - 1) * 4 + 4):
                        for nch in range(2):
                            pso = ps.tile([128, 512], F32, tag="psq",
                                          bufs=2, name="pso")
                            for k in range(HPAIRS - 1):
                                nc.tensor.matmul(
                                    pso,
                                    yT[k][:, tt * 128:(tt + 1) * 128],
                                    wpb[:, k, nch * 512:(nch + 1) * 512],
                                    start=(k == 0), stop=(k == HPAIRS - 2))
                                yield
                            pt = hsb.tile([128, 512], F32, tag="prj",
                                          bufs=8, name="prj")
                            nc.vector.tensor_copy(pt, pso)
                            proj3["parts"].append(pt)

                def proj3_fin():
                    """Tail: add hp3's contribution and store; evac on
                    the then-idle ScalarE-adjacent engines."""
                    ic = IC - 1
                    for i, tt in enumerate(range(ic * 4, ic * 4 + 4)):
                        ot = hsb.tile([128, 1024], BF16, tag="ot", bufs=3,
                                      name="ot")
                        for nch in range(2):
                            pso = ps.tile([128, 512], F32, tag="psq",
                                          bufs=2, name="pso")
                            nc.tensor.matmul(
                                pso, yT[3][:, tt * 128:(tt + 1) * 128],
                                wpb[:, 3, nch * 512:(nch + 1) * 512],
                                start=True, stop=True)
                            nc.vector.tensor_add(
                                ot[:, nch * 512:(nch + 1) * 512],
                                proj3["parts"][i * 2 + nch], pso)
                        nc.sync.dma_start(out[tt * 128:(tt + 1) * 128, :], ot)

                extras = []  # [deadline (hp, ic), generator]

                def pump(n):
                    done = 0
                    while extras and done < n:
                        try:
                            next(extras[0][1])
                            done += 1
                        except StopIteration:
                            extras.pop(0)

                def drain_due(pos):
                    i = 0
                    while i < len(extras):
                        if extras[i][0] <= pos:
                            for _ in extras[i][1]:
                                pass
                            extras.pop(i)
                        else:
                            i += 1

                pending = []

                def flush_pending():
                    for f in pending:
                        f()
                    pending.clear()

                # upfront: V for token tiles 0..3 and q/k(hp0) for tokens
                # 0..1023 — just enough for attention cell (ic0, hp0).
                # Everything else flows through the deadline queue, cell
                # order is ic-outer/hp-inner so projection chunks and V
                # groups spread across the whole exp-paced timeline.
                for _ in qk_gen(0, qts[0], kts[0], 0):
                    pass
                for _ in v_group(0):
                    pass
                for h in range(1, HPAIRS):
                    extras.append([(0, h), qk_gen(h, qts[h], kts[h], 0)])
                extras.append([(1, 0), v_group(1)])
                extras.append([(2, 0), v_group(2)])
                for h in range(HPAIRS):
                    extras.append([(2, h), qk_gen(h, qts[h], kts[h], 1)])
                extras.append([(3, 0), v_group(3)])

                for ic in range(IC):
                    for hp in range(HPAIRS):
                        qt, kt_t = qts[hp], kts[hp]
                        drain_due((ic, hp))
                        psys = [ps.tile([65, 512], F32, tag="psy", bufs=2,
                                        name=f"psy{h}") for h in range(2)]
                        # jt order: diagonal tdx=0 first (full width,
                        # start=True), then prior full tiles, then the
                        # shrunken diagonal suffix tiles.
                        jts = [4 * ic] + list(range(4 * ic)) + \
                            [4 * ic + 1, 4 * ic + 2, 4 * ic + 3]
                        for jn, jt in enumerate(jts):
                            tdx = jt - 4 * ic  # >=0 on diagonal tiles
                            sh = 128 * max(tdx, 0)   # query shrink offset
                            qs = slice(ic * 512 + sh, (ic + 1) * 512)
                            pss = ps.tile([128, 1024], F32, tag="pss",
                                          bufs=2, name="pss")
                            nc.tensor.matmul(
                                pss[:, sh:512],
                                kt_t[0:64, jt * 128:(jt + 1) * 128],
                                qt[0:64, qs], start=True, stop=True,
                                tile_position=(0, 0))
                            nc.tensor.matmul(
                                pss[:, 512 + sh:1024],
                                kt_t[64:128, jt * 128:(jt + 1) * 128],
                                qt[64:128, qs], start=True, stop=True,
                                tile_position=(64, 0))
                            et = hsb.tile([128, 1024], BF16, tag="et",
                                          bufs=5, name="et")
                            if sh:
                                pv = pss.rearrange("p (h c) -> p h c",
                                                   c=512)[:, :, sh:512]
                                ev = et.rearrange("p (h c) -> p h c",
                                                  c=512)[:, :, sh:512]
                            else:
                                pv, ev = pss, et
                            nc.scalar.activation(ev, pv, ActF.Exp,
                                                 scale=0.125)
                            if tdx >= 0:
                                nc.vector.tensor_mul(
                                    et[:, sh:sh + 128],
                                    et[:, sh:sh + 128], mask)
                                nc.vector.tensor_mul(
                                    et[:, 512 + sh:512 + sh + 128],
                                    et[:, 512 + sh:512 + sh + 128], mask)
                            for head in range(2):
                                vsl = v_sb[jt][:, (2 * hp + head) * 65:
                                               (2 * hp + head) * 65 + 65]
                                nc.tensor.matmul(
                                    psys[head][:, sh:512], vsl,
                                    et[:, head * 512 + sh:
                                       (head + 1) * 512],
                                    start=(jn == 0),
                                    stop=(jn == len(jts) - 1))
                            if jn == 2:
                                flush_pending()
                            pump(3)

                        # stage A: evacuate PSUM now (frees psys slots);
                        # Z row scattered over 128 partitions directly
                        yst = hsb.tile([128, 512], F32, tag="yst", bufs=2,
                                       name="yst")
                        zs = hsb.tile([128, 8], F32, tag="zs", bufs=2,
                                      name="zs")
                        for head in range(2):
                            t65 = hsb.tile([65, 512], F32, tag="t65",
                                           bufs=4, name="t65")
                            nc.vector.tensor_copy(t65, psys[head])
                            nc.sync.dma_start(
                                yst[head * 64:(head + 1) * 64, :],
                                t65[0:64, :])
                            nc.sync.dma_start(
                                zs[:, head * 4:(head + 1) * 4],
                                t65[64:65, :])

                        def stage_b(hp=hp, ic=ic, yst=yst, zs=zs):
                            # deferred: reciprocal, broadcast, normalize.
                            nc.vector.reciprocal(zs, zs)
                            zc = hsb.tile([1, 1024], F32, tag="zc", bufs=2,
                                          name="zc")
                            # per-head gathers whose [128,4] AP shape
                            # matches the scatter, so the element order
                            # round-trips exactly
                            nc.sync.dma_start(zc[0:1, 0:512], zs[:, 0:4])
                            nc.sync.dma_start(zc[0:1, 512:1024], zs[:, 4:8])
                            bcf = hsb.tile([128, 512], F32, tag="bcf",
                                           bufs=2, name="bcf")
                            nc.gpsimd.partition_broadcast(
                                bcf, zc[0:1, 512:1024])
                            nc.gpsimd.partition_broadcast(
                                bcf[0:64, :], zc[0:1, 0:512])
                            sl = slice(ic * 512, (ic + 1) * 512)
                            nc.vector.tensor_mul(yT[hp][:, sl], yst, bcf)
                            if hp == HPAIRS - 1:
                                extras.append(
                                    [(10 ** 9, 10 ** 9), proj_chunk(ic)])
                        if hp == HPAIRS - 1 and ic == IC - 1:
                            stage_b()
                        else:
                            pending.append(stage_b)
                # drain the tail: last normalize + projection of ic3
                flush_pending()
                pump(10 ** 9)
    nc.finalize()
    _nc_cache[key] = nc
    return nc


def make_in_maps(x, W_attn, b_attn, W_proj):
    """Build per-core input dicts from full inputs."""
    bf16 = ml_dtypes.bfloat16
    in_maps = []
    for c in range(N_CORES):
        b = c // 2
        g = c % 2
        cs = slice(g * 512, (g + 1) * 512)
        wb = np.stack([b_attn[0:D][cs], b_attn[D:2 * D][cs],
                       b_attn[2 * D:3 * D][cs]]).astype(np.float32)

        def wkc(m):  # [D, 512] -> [128, KT, 512]
            return np.ascontiguousarray(
                m.reshape(KT, 128, 512).transpose(1, 0, 2)).astype(bf16)

        # x[b].T [D, S] -> chunk-major [128, S//512, KT, 512]
        xt = x[b].T.reshape(KT, 128, S // 512, 512).transpose(1, 2, 0, 3)
        in_maps.append({
            "xT": np.ascontiguousarray(xt).astype(bf16),
            "Wq": wkc(W_attn[:, 0:D][:, cs]),
            "Wk": wkc(W_attn[:, D:2 * D][:, cs]),
            "Wv": wkc(W_attn[:, 2 * D:3 * D][:, cs]),
            "Wb": np.ascontiguousarray(wb),
            "Wp": np.ascontiguousarray(
                W_proj[cs, :].reshape(HPAIRS, 128, D).transpose(
                    1, 0, 2)).astype(bf16),
        })
    return in_maps


def kernel(x, W_attn, b_attn, W_proj, b_proj, trace=False):
    x = np.asarray(x, dtype=np.float32)
    W_attn = np.asarray(W_attn, dtype=np.float32)
    b_attn = np.asarray(b_attn, dtype=np.float32)
    W_proj = np.asarray(W_proj, dtype=np.float32)
    b_proj = np.asarray(b_proj, dtype=np.float32)
    nc = build_nc(x.shape[1], N_CORES)
    in_maps = make_in_maps(x, W_attn, b_attn, W_proj)
    res = bass_utils.run_bass_kernel_spmd(
        nc, in_maps, core_ids=list(range(N_CORES)), trace=trace)
    Bx, Sx, Dx = x.shape
    outp = np.empty((Bx, Sx, Dx), dtype=np.float32)
    for b in range(Bx):
        outp[b] = (np.asarray(res.results[2 * b]["out"], dtype=np.float32)
                   + np.asarray(res.results[2 * b + 1]["out"],
                                dtype=np.float32)
                   + b_proj[None, :])
    if trace:
        return outp, res
    return outp


# revision 38
# speedup vs baseline: 1.0173x; 1.0173x over previous
"""Causal self-attention kernel for 8 trn2 NeuronCores.

Sharding: 4 batches x 2 head-groups (8 heads each). Core c handles
batch c//2, heads (c%2)*8 .. (c%2)*8+8. Each core computes qkv for its
head-group, causal attention, and a partial projection; the host sums
the two head-group partials per batch and adds b_proj.

Design notes (370us baseline -> ~299us):
  - bf16 operands end-to-end (PSUM accumulate stays fp32); rel err
    ~3.8e-3 vs the 2e-2 gate.
  - fine-grained causal: diagonal 128-key tiles only compute the valid
    query suffix (scores, exp, PV all shrink); one 128x128 triangle
    mask handles the partial block.
  - attention runs ic-outer/hp-inner so every pipeline stage spreads
    across the exp(ScalarE)-paced timeline; V groups, q/k projections
    and the output projection all flow through a deadline-enforced
    deferred-work generator queue pumped between attention steps.
  - ScalarE runs ONLY exp; q/k PSUM evac is VectorE tensor_scalar_add,
    projection evac is VectorE copy.
  - the per-chunk softmax normalization (reciprocal/broadcast/mul) is
    emitted one chunk late so its DMA-latency chain never blocks the
    in-order Vector/GpSimd queues between chunks.
  - PSUM: pss 2x[128,1024] + psy 2x[65,512] + psq 2x[128,512] = 8 banks
    exactly; the psq pair double-buffers the whole deferred stream.
  - host pre-arranges x/weights into on-chip layouts so input DMAs are
    contiguous-row copies (the SP queue spends ~0.6us issuing each DMA
    and 1KB-run strided descriptors halve DMA engine throughput).
"""

import sys
import os

for _p in ("/opt/trn_rl_repo", "/root/.axon_site/_ro/trn_rl_repo"):
    if os.path.isdir(_p) and _p not in sys.path:
        sys.path.insert(0, _p)

import numpy as np
import ml_dtypes
import concourse.bass as bass  # noqa: F401
import concourse.mybir as mybir
import concourse.tile as tile
from concourse import bacc, bass_utils

F32 = mybir.dt.float32
F32R = mybir.dt.float32r
BF16 = mybir.dt.bfloat16
ActF = mybir.ActivationFunctionType

B, S, D, H = 4, 2048, 1024, 16
NH = 8          # heads per core
HPAIRS = NH // 2
KT = D // 128   # 8 k-tiles over D
N_CORES = 8

_nc_cache = {}


def build_nc(S_tok=S, n_cores=N_CORES):
    key = (S_tok, n_cores)
    if key in _nc_cache:
        return _nc_cache[key]
    IC = S_tok // 512      # query chunks
    NT = S_tok // 128      # token tiles
    nc = bacc.Bacc("TRN2", target_bir_lowering=False, debug=False,
                   num_devices=n_cores)
    # host pre-arranges x and weights into the on-chip layouts so every
    # input DMA is a long-contiguous-row copy (8KB runs)
    NCH = S_tok // 512
    xT = nc.dram_tensor("xT", [128, NCH, KT, 512], BF16,
                        kind="ExternalInput").ap()
    Wq = nc.dram_tensor("Wq", [128, KT, 512], BF16,
                        kind="ExternalInput").ap()
    Wk = nc.dram_tensor("Wk", [128, KT, 512], BF16,
                        kind="ExternalInput").ap()
    Wv = nc.dram_tensor("Wv", [128, KT, 512], BF16,
                        kind="ExternalInput").ap()
    Wb = nc.dram_tensor("Wb", [3, 512], F32, kind="ExternalInput").ap()
    Wp = nc.dram_tensor("Wp", [128, HPAIRS, D], BF16,
                        kind="ExternalInput").ap()
    out = nc.dram_tensor("out", [S_tok, D], BF16, kind="ExternalOutput").ap()

    with tile.TileContext(nc) as tc:
        with tc.tile_pool(name="persist", bufs=1) as pp:
            # big resident tiles; [p, k, cols] so one strided DMA loads all
            xtr = pp.tile([128, NCH, KT, 512], BF16, name="xtr")
            wvb = pp.tile([128, KT, 512], BF16, name="wvb")
            wqb = pp.tile([128, KT, 512], BF16, name="wqb")
            wkb = pp.tile([128, KT, 512], BF16, name="wkb")
            wpb = pp.tile([128, HPAIRS, D], BF16, name="wpb")
            # v in natural layout, 65-stride per head (64 v cols + ones col)
            v_sb = [pp.tile([128, 8 * 65], BF16, name=f"vsb{t}")
                    for t in range(NT)]
            # normalized attention output per head pair [local d, tokens]
            yT = [pp.tile([128, S_tok], BF16, name=f"ytr{h}")
                  for h in range(HPAIRS)]
            # triangle mask: m[p, y] = 1 if y >= p else 0
            mask = pp.tile([128, 128], BF16, name="mask")
            # rank-1 stationary for broadcasting 1/Z rows across partitions
            ones64 = pp.tile([1, 64], F32, name="ones64")
            nc.gpsimd.memset(ones64, 1.0)
            nc.gpsimd.memset(mask, 1.0)
            nc.gpsimd.affine_select(
                out=mask, in_=mask, compare_op=mybir.AluOpType.is_ge,
                fill=0.0, base=0, pattern=[[1, 128]],
                channel_multiplier=-1)
            # softmax-denominator ones columns, written once
            for t in range(NT):
                nc.gpsimd.memset(
                    v_sb[t].rearrange("p (h c) -> p h c", c=65)[:, :, 64:65],
                    1.0)

            # ---- input DMAs: host-prearranged layouts make every load
            # a contiguous-row copy (8KB runs); chunk-0 of x and the V
            # weights first since they gate the first matmul; the v bias
            # row early so its broadcast never blocks the V evacs ----
            nc.sync.dma_start(xtr[:, 0, :, :], xT[:, 0, :, :])
            nc.sync.dma_start(wvb, Wv)
            bvr = pp.tile([1, 512], F32, name="bvr")
            nc.sync.dma_start(bvr, Wb[2:3, :])
            bvb = pp.tile([128, 512], F32, name="bvb")
            nc.gpsimd.partition_broadcast(bvb, bvr)
            nc.sync.dma_start(xtr[:, 1, :, :], xT[:, 1, :, :])
            wq9, wk9 = [], []
            for hp in range(HPAIRS):
                hs = slice(hp * 128, (hp + 1) * 128)
                t9 = pp.tile([128, 1], F32, name=f"wq9_{hp}")
                nc.sync.dma_start(t9, Wb[0:1, hs])
                wq9.append(t9)
                t9 = pp.tile([128, 1], F32, name=f"wk9_{hp}")
                nc.sync.dma_start(t9, Wb[1:2, hs])
                wk9.append(t9)
            nc.sync.dma_start(wqb, Wq)
            nc.sync.dma_start(wkb, Wk)
            nc.sync.dma_start(xtr[:, 2, :, :], xT[:, 2, :, :])
            nc.sync.dma_start(xtr[:, 3, :, :], xT[:, 3, :, :])
            nc.sync.dma_start(wpb, Wp)

            # ---- attention with V / q-k / projection all flowing through
            # a deadline-enforced deferred-work queue ----
            with tc.tile_pool(name="hsb", bufs=1) as hsb, \
                 tc.tile_pool(name="ps", bufs=1, space="PSUM") as ps:

                def v_group(g):
                    """Generator: V projection + bias for token tiles
                    4g..4g+3, one [128,512] psq slot per token tile
                    (bufs=2 hides the evac latency)."""
                    for t in range(g * 4, g * 4 + 4):
                        psv = ps.tile([128, 512], F32, tag="psq", bufs=2,
                                      name="psv")
                        for k in range(KT):
                            nc.tensor.matmul(
                                psv, xtr[:, t // 4, k,
                                     (t % 4) * 128:(t % 4) * 128 + 128],
                                wvb[:, k, :], start=(k == 0),
                                stop=(k == KT - 1))
                            yield
                        vv = v_sb[t].rearrange("p (h c) -> p h c", c=65)
                        nc.vector.tensor_add(
                            vv[:, :, 0:64],
                            psv.rearrange("p (h c) -> p h c", c=64),
                            bvb.rearrange("p (h c) -> p h c", c=64))

                # all four head-pairs' q/k stay live (bf16 keeps it small)
                qts = [hsb.tile([128, S_tok], BF16, name=f"qt{h}")
                       for h in range(HPAIRS)]
                kts = [hsb.tile([128, S_tok], BF16, name=f"kt{h}")
                       for h in range(HPAIRS)]

                def qk_gen(hp, qt, kt_t, half):
                    """Generator: q/k projection (one token half) for
                    head-pair hp; evac on VectorE with per-partition
                    bias, one [128,512] psq slot per 512-token chunk."""
                    hs = slice(hp * 128, (hp + 1) * 128)
                    for dst, wb_, w9 in ((qt, wqb, wq9[hp]),
                                         (kt_t, wkb, wk9[hp])):
                        for sub in range(2):
                            ch = half * 2 + sub
                            psq = ps.tile([128, 512], F32, tag="psq",
                                          bufs=2, name="psq")
                            for k in range(KT):
                                nc.tensor.matmul(
                                    psq, wb_[:, k, hs],
                                    xtr[:, ch, k, :],
                                    start=(k == 0), stop=(k == KT - 1))
                                yield
                            nc.vector.tensor_scalar_add(
                                dst[:, ch * 512:(ch + 1) * 512], psq, w9)

                def proj_chunk(ic):
                    """Generator: output projection for token tiles of
                    query chunk ic; needs yT[*][:, ic*512:(ic+1)*512]."""
                    for tt in range(ic * 4, ic * 4 + 4):
                        ot = hsb.tile([128, 1024], BF16, tag="ot", bufs=3,
                                      name="ot")
                        for nch in range(2):
                            pso = ps.tile([128, 512], F32, tag="psq",
                                          bufs=2, name="pso")
                            for k in range(HPAIRS):
                                nc.tensor.matmul(
                                    pso,
                                    yT[k][:, tt * 128:(tt + 1) * 128],
                                    wpb[:, k, nch * 512:(nch + 1) * 512],
                                    start=(k == 0), stop=(k == HPAIRS - 1))
                                yield
                            nc.vector.tensor_copy(
                                ot[:, nch * 512:(nch + 1) * 512], pso)
                        nc.sync.dma_start(out[tt * 128:(tt + 1) * 128, :], ot)

                proj3 = {"parts": []}

                def proj3_pre():
                    """Generator: last-chunk projection partials over
                    head-pairs 0..2 (hp3's yT is not ready yet); runs
                    during the last cell's attention."""
                    for tt in range((IC - 1) * 4, (IC - 1) * 4 + 4):
                        for nch in range(2):
                            pso = ps.tile([128, 512], F32, tag="psq",
                                          bufs=2, name="pso")
                            for k in range(HPAIRS - 1):
                                nc.tensor.matmul(
                                    pso,
                                    yT[k][:, tt * 128:(tt + 1) * 128],
                                    wpb[:, k, nch * 512:(nch + 1) * 512],
                                    start=(k == 0), stop=(k == HPAIRS - 2))
                                yield
                            pt = hsb.tile([128, 512], F32, tag="prj",
                                          bufs=8, name="prj")
                            nc.vector.tensor_copy(pt, pso)
                            proj3["parts"].append(pt)

                def proj3_fin():
                    """Tail: add hp3's contribution and store; evac on
                    the then-idle ScalarE-adjacent engines."""
                    ic = IC - 1
                    for i, tt in enumerate(range(ic * 4, ic * 4 + 4)):
                        ot = hsb.tile([128, 1024], BF16, tag="ot", bufs=3,
                                      name="ot")
                        for nch in range(2):
                            pso = ps.tile([128, 512], F32, tag="psq",
                                          bufs=2, name="pso")
                            nc.tensor.matmul(
                                pso, yT[3][:, tt * 128:(tt + 1) * 128],
                                wpb[:, 3, nch * 512:(nch + 1) * 512],
                                start=True, stop=True)
                            nc.vector.tensor_add(
                                ot[:, nch * 512:(nch + 1) * 512],
                                proj3["parts"][i * 2 + nch], pso)
                        nc.sync.dma_start(out[tt * 128:(tt + 1) * 128, :], ot)

                extras = []  # [deadline (hp, ic), generator]

                def pump(n):
                    done = 0
                    while extras and done < n:
                        try:
                            next(extras[0][1])
                            done += 1
                        except StopIteration:
                            extras.pop(0)

                def drain_due(pos):
                    i = 0
                    while i < len(extras):
                        if extras[i][0] <= pos:
                            for _ in extras[i][1]:
                                pass
                            extras.pop(i)
                        else:
                            i += 1

                pending = []

                def flush_pending():
                    for f in pending:
                        f()
                    pending.clear()

                # upfront: V for token tiles 0..3 and q/k(hp0) for tokens
                # 0..1023 — just enough for attention cell (ic0, hp0).
                # Everything else flows through the deadline queue, cell
                # order is ic-outer/hp-inner so projection chunks and V
                # groups spread across the whole exp-paced timeline.
                for _ in v_group(0):
                    pass
                for _ in qk_gen(0, qts[0], kts[0], 0):
                    pass
                for h in range(1, HPAIRS):
                    extras.append([(0, h), qk_gen(h, qts[h], kts[h], 0)])
                extras.append([(1, 0), v_group(1)])
                extras.append([(2, 0), v_group(2)])
                for h in range(HPAIRS):
                    extras.append([(2, h), qk_gen(h, qts[h], kts[h], 1)])
                extras.append([(3, 0), v_group(3)])

                for ic in range(IC):
                    for hp in range(HPAIRS):
                        qt, kt_t = qts[hp], kts[hp]
                        drain_due((ic, hp))
                        psys = [ps.tile([65, 512], F32, tag="psy", bufs=2,
                                        name=f"psy{h}") for h in range(2)]
                        # jt order: diagonal tdx=0 first (full width,
                        # start=True), then prior full tiles, then the
                        # shrunken diagonal suffix tiles.
                        jts = [4 * ic] + list(range(4 * ic)) + \
                            [4 * ic + 1, 4 * ic + 2, 4 * ic + 3]
                        for jn, jt in enumerate(jts):
                            tdx = jt - 4 * ic  # >=0 on diagonal tiles
                            sh = 128 * max(tdx, 0)   # query shrink offset
                            qs = slice(ic * 512 + sh, (ic + 1) * 512)
                            pss = ps.tile([128, 1024], F32, tag="pss",
                                          bufs=2, name="pss")
                            nc.tensor.matmul(
                                pss[:, sh:512],
                                kt_t[0:64, jt * 128:(jt + 1) * 128],
                                qt[0:64, qs], start=True, stop=True,
                                tile_position=(0, 0))
                            nc.tensor.matmul(
                                pss[:, 512 + sh:1024],
                                kt_t[64:128, jt * 128:(jt + 1) * 128],
                                qt[64:128, qs], start=True, stop=True,
                                tile_position=(64, 0))
                            et = hsb.tile([128, 1024], BF16, tag="et",
                                          bufs=5, name="et")
                            if sh:
                                pv = pss.rearrange("p (h c) -> p h c",
                                                   c=512)[:, :, sh:512]
                                ev = et.rearrange("p (h c) -> p h c",
                                                  c=512)[:, :, sh:512]
                            else:
                                pv, ev = pss, et
                            nc.scalar.activation(ev, pv, ActF.Exp,
                                                 scale=0.125)
                            if tdx >= 0:
                                nc.vector.tensor_mul(
                                    et[:, sh:sh + 128],
                                    et[:, sh:sh + 128], mask)
                                nc.vector.tensor_mul(
                                    et[:, 512 + sh:512 + sh + 128],
                                    et[:, 512 + sh:512 + sh + 128], mask)
                            for head in range(2):
                                vsl = v_sb[jt][:, (2 * hp + head) * 65:
                                               (2 * hp + head) * 65 + 65]
                                nc.tensor.matmul(
                                    psys[head][:, sh:512], vsl,
                                    et[:, head * 512 + sh:
                                       (head + 1) * 512],
                                    start=(jn == 0),
                                    stop=(jn == len(jts) - 1))
                            if jn == 2:
                                flush_pending()
                            pump(3)

                        # stage A: evacuate PSUM now (frees psys slots);
                        # Z row scattered over 128 partitions directly
                        yst = hsb.tile([128, 512], F32, tag="yst", bufs=2,
                                       name="yst")
                        zs = hsb.tile([128, 8], F32, tag="zs", bufs=2,
                                      name="zs")
                        for head in range(2):
                            t65 = hsb.tile([65, 512], F32, tag="t65",
                                           bufs=4, name="t65")
                            nc.vector.tensor_copy(t65, psys[head])
                            nc.sync.dma_start(
                                yst[head * 64:(head + 1) * 64, :],
                                t65[0:64, :])
                            nc.sync.dma_start(
                                zs[:, head * 4:(head + 1) * 4],
                                t65[64:65, :])

                        def stage_b(hp=hp, ic=ic, yst=yst, zs=zs):
                            # deferred: reciprocal, broadcast, normalize.
                            nc.vector.reciprocal(zs, zs)
                            zc = hsb.tile([1, 1024], F32, tag="zc", bufs=2,
                                          name="zc")
                            # per-head gathers whose [128,4] AP shape
                            # matches the scatter, so the element order
                            # round-trips exactly
                            nc.sync.dma_start(zc[0:1, 0:512], zs[:, 0:4])
                            nc.sync.dma_start(zc[0:1, 512:1024], zs[:, 4:8])
                            bcf = hsb.tile([128, 512], F32, tag="bcf",
                                           bufs=2, name="bcf")
                            nc.gpsimd.partition_broadcast(
                                bcf, zc[0:1, 512:1024])
                            nc.gpsimd.partition_broadcast(
                                bcf[0:64, :], zc[0:1, 0:512])
                            sl = slice(ic * 512, (ic + 1) * 512)
                            nc.vector.tensor_mul(yT[hp][:, sl], yst, bcf)
                            if hp == HPAIRS - 1:
                                extras.append(
                                    [(10 ** 9, 10 ** 9), proj_chunk(ic)])
                        if hp == HPAIRS - 1 and ic == IC - 1:
                            stage_b()
                        else:
                            pending.append(stage_b)
                # drain the tail: last normalize + projection of ic3
                flush_pending()
                pump(10 ** 9)
    nc.finalize()
    _nc_cache[key] = nc
    return nc


def make_in_maps(x, W_attn, b_attn, W_proj):
    """Build per-core input dicts from full inputs."""
    bf16 = ml_dtypes.bfloat16
    in_maps = []
    for c in range(N_CORES):
        b = c // 2
        g = c % 2
        cs = slice(g * 512, (g + 1) * 512)
        wb = np.stack([b_attn[0:D][cs], b_attn[D:2 * D][cs],
                       b_attn[2 * D:3 * D][cs]]).astype(np.float32)

        def wkc(m):  # [D, 512] -> [128, KT, 512]
            return np.ascontiguousarray(
                m.reshape(KT, 128, 512).transpose(1, 0, 2)).astype(bf16)

        # x[b].T [D, S] -> chunk-major [128, S//512, KT, 512]
        xt = x[b].T.reshape(KT, 128, S // 512, 512).transpose(1, 2, 0, 3)
        in_maps.append({
            "xT": np.ascontiguousarray(xt).astype(bf16),
            "Wq": wkc(W_attn[:, 0:D][:, cs]),
            "Wk": wkc(W_attn[:, D:2 * D][:, cs]),
            "Wv": wkc(W_attn[:, 2 * D:3 * D][:, cs]),
            "Wb": np.ascontiguousarray(wb),
            "Wp": np.ascontiguousarray(
                W_proj[cs, :].reshape(HPAIRS, 128, D).transpose(
                    1, 0, 2)).astype(bf16),
        })
    return in_maps


def kernel(x, W_attn, b_attn, W_proj, b_proj, trace=False):
    x = np.asarray(x, dtype=np.float32)
    W_attn = np.asarray(W_attn, dtype=np.float32)
    b_attn = np.asarray(b_attn, dtype=np.float32)
    W_proj = np.asarray(W_proj, dtype=np.float32)
    b_proj = np.asarray(b_proj, dtype=np.float32)
    nc = build_nc(x.shape[1], N_CORES)
    in_maps = make_in_maps(x, W_attn, b_attn, W_proj)
    res = bass_utils.run_bass_kernel_spmd(
        nc, in_maps, core_ids=list(range(N_CORES)), trace=trace)
    Bx, Sx, Dx = x.shape
    outp = np.empty((Bx, Sx, Dx), dtype=np.float32)
    for b in range(Bx):
        outp[b] = (np.asarray(res.results[2 * b]["out"], dtype=np.float32)
                   + np.asarray(res.results[2 * b + 1]["out"],
                                dtype=np.float32)
                   + b_proj[None, :])
    if trace:
        return outp, res
    return outp
